# revision 6
# baseline (speedup 1.0000x reference)
"""Trainium2 Bass kernel for nn_DecoupledAttentionWeight.

Computes the five projections q_sem/k_sem/q_geo/k_geo/v of x, applies RoPE to
the geo paths, the per-head sigmoid gate + per-path scaling to q (folded into
the projection weights host-side), and returns (q_cat, k_cat, vh) shaped
(B, H, T, 128) each.

Sharding over 8 NeuronCores: 2-way data-parallel over batch (batches {0,1} /
{2,3}) x 4-way tensor-parallel over heads (4 heads per core). Each core runs
one big [8192 x 2048] @ [2048 x 1536] matmul in bf16 (full PE speed, ~2.3e-3
rel err, well within tolerance) with the per-head output columns packed as
[q_sem|q_geo|k_sem|k_geo|v] so the sem||geo concat is free, then RoPE on the
geo strips via DVE with broadcast access patterns.

v3 changes vs v2 (702us):
- cos/sin tables pre-permuted on host to the SBUF layout -> one big-descriptor
  DMA instead of 4096x128B descriptors that hogged the scalar sequencer 20us.
- Startup wavefront: m-tiles 0 and 1 interleaved per k so PE consumption
  (1.3us/k-tile) matches the W DMA arrival pace (~1.3us/k-tile) instead of
  racing ahead and stalling.
- Last m-tile runs chunk-outer with per-head postprocess + per-head output
  DMAs so the drain tail shrinks from ~9us to ~4us.
"""
import math
import os
import sys

import numpy as np

for _p in ("/opt/trn_rl_repo", os.path.expanduser("~/.axon_site/_ro/trn_rl_repo")):
    if os.path.isdir(_p) and _p not in sys.path:
        sys.path.insert(0, _p)

import concourse.bacc as bacc
import concourse.mybir as mybir
import concourse.tile as tile
from concourse.bass_utils import run_bass_kernel_spmd

# Problem config (hardcoded from the nn.Module init)
D_MODEL = 2048
N_HEADS = 16
SEM_HD = 64
GEO_HD = 64
HEAD_DIM = 128
ROPE_DIM = 64
ROPE_HALF = ROPE_DIM // 2  # 32
ROPE_BASE = 10000.0
B, T = 4, 4096

# Sharding: 2 row groups (2 batches each) x 4 head groups (4 heads each)
N_CORES = 8
RG, HG = 2, 4
ROWS_PER_CORE = (B * T) // RG          # 8192
HEADS_PER_CORE = N_HEADS // HG         # 4
BLK = SEM_HD + GEO_HD + SEM_HD + GEO_HD + HEAD_DIM  # 384 cols per head
N_CORE = HEADS_PER_CORE * BLK          # 1536
K_TILES = D_MODEL // 128               # 16
M_TILES = ROWS_PER_CORE // 128         # 64
SLAB_MT = 2                            # m_tiles per input DMA slab
SLAB_ROWS = SLAB_MT * 128              # 256
N_SLABS = M_TILES // SLAB_MT           # 32
CHUNK = 512                            # psum bank / matmul moving size
N_CHUNKS = N_CORE // CHUNK             # 3
COS_SLOTS = T // 128                   # 32 distinct cos/sin row-tiles

_f32 = mybir.dt.float32
_bf16 = mybir.dt.bfloat16


def _build_nc():
    nc = bacc.Bacc("TRN2", target_bir_lowering=False, debug=False, num_devices=1)
    xt_d = nc.dram_tensor("xt", [D_MODEL, ROWS_PER_CORE], _bf16, kind="ExternalInput")
    w_d = nc.dram_tensor("w", [D_MODEL, N_CORE], _bf16, kind="ExternalInput")
    # host pre-permuted to the SBUF-resident layout [128, slot*half]
    cos_d = nc.dram_tensor("cos", [128, COS_SLOTS * ROPE_HALF], _f32,
                           kind="ExternalInput")
    sin_d = nc.dram_tensor("sin", [128, COS_SLOTS * ROPE_HALF], _f32,
                           kind="ExternalInput")
    qkv_d = nc.dram_tensor(
        "qkv", [3, HEADS_PER_CORE, ROWS_PER_CORE, HEAD_DIM], _f32,
        kind="ExternalOutput",
    )

    with tile.TileContext(nc) as tc:
        with (
            tc.tile_pool(name="wp", bufs=1) as wp,
            tc.tile_pool(name="xp", bufs=3) as xp,
            tc.tile_pool(name="trig", bufs=1) as trigp,
            tc.tile_pool(name="stg", bufs=3) as stgp,
            tc.tile_pool(name="tmp", bufs=2) as tmpp,
            tc.tile_pool(name="ps", bufs=2, space="PSUM") as ps,
        ):
            xt_kd = xt_d.ap().rearrange("(k p) m -> p k m", p=128)
            slab_tiles = {}

            def load_slab(s, split=1):
                if s not in slab_tiles:
                    t = xp.tile([128, K_TILES * SLAB_ROWS], _bf16, tag="xt")
                    view = t[:].rearrange("p (k m) -> p k m", k=K_TILES)
                    kk = K_TILES // split
                    for j in range(split):
                        # scalar HWDGE ring: parallel to W/output DMAs
                        nc.scalar.dma_start(
                            view[:, j * kk:(j + 1) * kk, :],
                            xt_kd[:, j * kk:(j + 1) * kk,
                                  s * SLAB_ROWS:(s + 1) * SLAB_ROWS],
                        )
                    slab_tiles[s] = t
                return slab_tiles[s]

            # First x slab split in 4 so the k=0 matmul can start after ~256KB.
            load_slab(0, split=4)

            # Weights resident, one tile per k, alternating two DMA rings.
            w_kd = w_d.ap().rearrange("(k p) n -> k p n", p=128)
            w_tiles = []
            for k in range(K_TILES):
                wt = wp.tile([128, N_CORE], _bf16, tag=f"w{k}")
                eng = nc.sync if k % 2 == 0 else nc.gpsimd
                eng.dma_start(wt[:], w_kd[k])
                w_tiles.append(wt)

            # Prefetch slab 1 before the trig tables so it isn't queued
            # behind them on the scalar ring.
            load_slab(1)

            # cos/sin tables resident (already in SBUF layout).
            cos_sb = trigp.tile([128, COS_SLOTS * ROPE_HALF], _f32, tag="cos")
            nc.scalar.dma_start(cos_sb[:], cos_d.ap())
            sin_sb = trigp.tile([128, COS_SLOTS * ROPE_HALF], _f32, tag="sin")
            nc.scalar.dma_start(sin_sb[:], sin_d.ap())
            cos_v = cos_sb[:].rearrange("p (s c) -> p s c", s=COS_SLOTS)
            sin_v = sin_sb[:].rearrange("p (s c) -> p s c", s=COS_SLOTS)

            qkv_v = qkv_d.ap()

            def emit_mms(xt_v, i, psum, k):
                """3 chunk matmuls for (m-tile i-within-slab, k)."""
                lhs = xt_v[:, k, i * 128:(i + 1) * 128]
                for c in range(N_CHUNKS):
                    inst = nc.tensor.matmul(
                        psum[:, c * CHUNK:(c + 1) * CHUNK],
                        lhs,
                        w_tiles[k][:, c * CHUNK:(c + 1) * CHUNK],
                        start=(k == 0),
                        stop=(k == K_TILES - 1),
                    )
                    if c > 0:
                        inst.ldweights = False

            def emit_post(psum, mt, stg, hs, he):
                """RoPE + copies for heads [hs, he) of one m-tile."""
                pv = psum[:, :].rearrange(
                    "p (h t c) -> p h t c", h=HEADS_PER_CORE, t=3
                )
                sv = stg[:].rearrange(
                    "p (t h c) -> p h t c", h=HEADS_PER_CORE, t=3
                )
                nh = he - hs
                slot = mt % COS_SLOTS
                cos_bc = (
                    cos_v[:, slot, :]
                    .unsqueeze(1)
                    .unsqueeze(1)
                    .broadcast_to([128, nh, 2, ROPE_HALF])
                )
                sin_bc = (
                    sin_v[:, slot, :]
                    .unsqueeze(1)
                    .unsqueeze(1)
                    .broadcast_to([128, nh, 2, ROPE_HALF])
                )
                x1 = pv[:, hs:he, 0:2, 64:96]
                x2 = pv[:, hs:he, 0:2, 96:128]
                shp = [128, nh, 2, ROPE_HALF]
                t1 = tmpp.tile(shp, _f32, tag="t1")
                t2 = tmpp.tile(shp, _f32, tag="t2")
                t3 = tmpp.tile(shp, _f32, tag="t3")
                t4 = tmpp.tile(shp, _f32, tag="t4")
                nc.vector.tensor_mul(t1[:], x1, cos_bc)
                nc.vector.tensor_mul(t2[:], x2, sin_bc)
                nc.vector.tensor_mul(t3[:], x2, cos_bc)
                nc.vector.tensor_mul(t4[:], x1, sin_bc)
                nc.vector.tensor_sub(sv[:, hs:he, 0:2, 64:96], t1[:], t2[:])
                nc.vector.tensor_add(sv[:, hs:he, 0:2, 96:128], t3[:], t4[:])
                # sem halves of q and k
                nc.any.tensor_copy(
                    sv[:, hs:he, 0:2, 0:64], pv[:, hs:he, 0:2, 0:64]
                )
                # v
                nc.any.tensor_copy(sv[:, hs:he, 2, :], pv[:, hs:he, 2, :])

            def emit_out(stg, mt):
                m0 = mt * 128
                nc.sync.dma_start(
                    qkv_v[:, :, m0:m0 + 128, :].transpose([2, 0, 1, 3]),
                    stg[:].rearrange(
                        "p (t h c) -> p t h c", h=HEADS_PER_CORE, t=3
                    ),
                )

            def emit_out_head(stg, mt, h):
                m0 = mt * 128
                nc.sync.dma_start(
                    qkv_v[:, h, m0:m0 + 128, :].transpose([1, 0, 2]),
                    stg[:].rearrange(
                        "p (t h c) -> p t h c", h=HEADS_PER_CORE, t=3
                    )[:, :, h, :],
                )

            # ---- Startup wavefront: m-tiles 0 and 1 interleaved per k so the
            # PE consumes W k-tiles at the pace they arrive from HBM.
            xt_v0 = load_slab(0)[:].rearrange("p (k m) -> p k m", k=K_TILES)
            psum_a = ps.tile([128, N_CORE], _f32, name="psum", tag="psum")
            psum_b = ps.tile([128, N_CORE], _f32, name="psum", tag="psum")
            for k in range(K_TILES):
                emit_mms(xt_v0, 0, psum_a, k)
                emit_mms(xt_v0, 1, psum_b, k)
            for mt, psum in ((0, psum_a), (1, psum_b)):
                stg = stgp.tile([128, N_CORE], _f32, tag="stg")
                emit_post(psum, mt, stg, 0, HEADS_PER_CORE)
                emit_out(stg, mt)

            # ---- Steady state: m-tiles 2..62
            for s in range(1, N_SLABS):
                xt_sb = load_slab(s)
                xt_v = xt_sb[:].rearrange("p (k m) -> p k m", k=K_TILES)
                for i in range(SLAB_MT):
                    mt = s * SLAB_MT + i
                    if mt == M_TILES - 1:
                        break
                    psum = ps.tile([128, N_CORE], _f32, name="psum", tag="psum")
                    for k in range(K_TILES):
                        emit_mms(xt_v, i, psum, k)
                    stg = stgp.tile([128, N_CORE], _f32, tag="stg")
                    emit_post(psum, mt, stg, 0, HEADS_PER_CORE)
                    emit_out(stg, mt)

            # ---- Last m-tile: chunk-outer so heads complete progressively;
            # per-head postprocess + output DMA shrink the serial tail.
            mt = M_TILES - 1
            i = mt % SLAB_MT
            xt_v = slab_tiles[N_SLABS - 1][:].rearrange(
                "p (k m) -> p k m", k=K_TILES
            )
            psum = ps.tile([128, N_CORE], _f32, name="psum", tag="psum")
            lhs_col = lambda k: xt_v[:, k, i * 128:(i + 1) * 128]
            stg = stgp.tile([128, N_CORE], _f32, tag="stg")
            for c in range(N_CHUNKS):
                for k in range(K_TILES):
                    nc.tensor.matmul(
                        psum[:, c * CHUNK:(c + 1) * CHUNK],
                        lhs_col(k),
                        w_tiles[k][:, c * CHUNK:(c + 1) * CHUNK],
                        start=(k == 0),
                        stop=(k == K_TILES - 1),
                    )
                if c == 1:
                    # heads 0,1 live in cols 0..767 = chunks 0,1
                    emit_post(psum, mt, stg, 0, 2)
                    emit_out_head(stg, mt, 0)
                    emit_out_head(stg, mt, 1)
            emit_post(psum, mt, stg, 2, 4)
            emit_out_head(stg, mt, 2)
            emit_out_head(stg, mt, 3)

    nc.compile()
    return nc


_NC_CACHE = None
LAST_RESULTS = None


def _get_nc():
    global _NC_CACHE
    if _NC_CACHE is None:
        _NC_CACHE = _build_nc()
    return _NC_CACHE


def _host_tables(pos_offset):
    """cos/sin tables computed exactly as the reference does (f32 jax ops),
    pre-permuted to the SBUF layout [p, slot*half] with row index p holding
    positions {slot*128 + p}."""
    import jax
    import jax.numpy as jnp

    with jax.default_device(jax.devices("cpu")[0]):
        inv_freq = ROPE_BASE ** (
            -jnp.arange(0, ROPE_HALF, dtype=jnp.float32) * (2.0 / ROPE_DIM)
        )
        pos = jnp.arange(T, dtype=jnp.float32) + jnp.float32(pos_offset)
        ang = pos[:, None] * inv_freq[None, :]
        cos = np.asarray(jnp.cos(ang), dtype=np.float32)
        sin = np.asarray(jnp.sin(ang), dtype=np.float32)

    def permute(a):  # (T, half) -> (128, slots*half)
        return np.ascontiguousarray(
            a.reshape(COS_SLOTS, 128, ROPE_HALF)
            .transpose(1, 0, 2)
            .reshape(128, COS_SLOTS * ROPE_HALF)
        )

    return permute(cos), permute(sin)


def _gate(gate_logit):
    import jax
    import jax.numpy as jnp

    with jax.default_device(jax.devices("cpu")[0]):
        g = np.asarray(
            jax.nn.sigmoid(jnp.asarray(gate_logit, dtype=jnp.float32)),
            dtype=np.float32,
        )
    return g


def kernel(x, wq_sem, wk_sem, wq_geo, wk_geo, wv, gate_logit, pos_offset):
    import ml_dtypes

    bf16 = ml_dtypes.bfloat16
    x = np.asarray(x, dtype=np.float32)
    wq_sem = np.asarray(wq_sem, dtype=np.float32)
    wk_sem = np.asarray(wk_sem, dtype=np.float32)
    wq_geo = np.asarray(wq_geo, dtype=np.float32)
    wk_geo = np.asarray(wk_geo, dtype=np.float32)
    wv = np.asarray(wv, dtype=np.float32)
    pos_off = int(np.asarray(pos_offset))

    g = _gate(gate_logit)  # (16,)
    sem_scale = np.float32(1.0 / math.sqrt(float(SEM_HD)))
    geo_scale = np.float32(1.0 / math.sqrt(float(GEO_HD)))
    q_sem_col = (np.float32(2.0) * g * sem_scale).astype(np.float32)   # per head
    q_geo_col = ((np.float32(2.0) - np.float32(2.0) * g) * geo_scale).astype(
        np.float32
    )

    # Per-core weight slabs, cols per head: [qsem|qgeo|ksem|kgeo|v]
    w_cores = []
    for hg in range(HG):
        cols = []
        for hl in range(HEADS_PER_CORE):
            h = hg * HEADS_PER_CORE + hl
            cols.append(wq_sem[:, h * 64:(h + 1) * 64] * q_sem_col[h])
            cols.append(wq_geo[:, h * 64:(h + 1) * 64] * q_geo_col[h])
            cols.append(wk_sem[:, h * 64:(h + 1) * 64])
            cols.append(wk_geo[:, h * 64:(h + 1) * 64])
            cols.append(wv[:, h * 128:(h + 1) * 128])
        w_cores.append(np.concatenate(cols, axis=1).astype(bf16))

    # x^T in bf16, split into the two row groups
    xt = x.reshape(B * T, D_MODEL).T.astype(bf16)  # (2048, 16384) C-contig copy
    xt_rg = [
        np.ascontiguousarray(xt[:, rg * ROWS_PER_CORE:(rg + 1) * ROWS_PER_CORE])
        for rg in range(RG)
    ]

    cos, sin = _host_tables(pos_off)

    in_maps = []
    for core in range(N_CORES):
        rg, hg = core // HG, core % HG
        in_maps.append(
            {"xt": xt_rg[rg], "w": w_cores[hg], "cos": cos, "sin": sin}
        )

    nc = _get_nc()
    res = run_bass_kernel_spmd(nc, in_maps, list(range(N_CORES)))
    global LAST_RESULTS
    LAST_RESULTS = res

    q_cat = np.empty((B, N_HEADS, T, HEAD_DIM), np.float32)
    k_cat = np.empty((B, N_HEADS, T, HEAD_DIM), np.float32)
    vh = np.empty((B, N_HEADS, T, HEAD_DIM), np.float32)
    for core in range(N_CORES):
        rg, hg = core // HG, core % HG
        a = res.results[core]["qkv"]  # (3, 4, 8192, 128)
        for t3_idx, dst in ((0, q_cat), (1, k_cat), (2, vh)):
            # (4, 8192, 128) -> (heads, b_local, T, 128)
            b = a[t3_idx].reshape(HEADS_PER_CORE, 2, T, HEAD_DIM)
            dst[
                rg * 2:(rg + 1) * 2,
                hg * HEADS_PER_CORE:(hg + 1) * HEADS_PER_CORE,
            ] = b.transpose(1, 0, 2, 3)
    return q_cat, k_cat, vh


# revision 7
# speedup vs baseline: 1.2004x; 1.2004x over previous
"""Trainium2 Bass kernel for nn_DecoupledAttentionWeight.

Computes the five projections q_sem/k_sem/q_geo/k_geo/v of x, applies RoPE to
the geo paths, the per-head sigmoid gate + per-path scaling to q (folded into
the projection weights host-side), and returns (q_cat, k_cat, vh) shaped
(B, H, T, 128) each.

Sharding over 8 NeuronCores: 2-way data-parallel over batch (batches {0,1} /
{2,3}) x 4-way tensor-parallel over heads (4 heads per core). Each core runs
one big [8192 x 2048] @ [2048 x 1536] matmul in bf16 (full PE speed, ~2.3e-3
rel err, well within tolerance) with the per-head output columns packed as
[q_sem|q_geo|k_sem|k_geo|v] so the sem||geo concat is free, then RoPE on the
geo strips via DVE with broadcast access patterns.

v3 changes vs v2 (702us):
- cos/sin tables pre-permuted on host to the SBUF layout -> one big-descriptor
  DMA instead of 4096x128B descriptors that hogged the scalar sequencer 20us.
- Startup wavefront: m-tiles 0 and 1 interleaved per k so PE consumption
  (1.3us/k-tile) matches the W DMA arrival pace (~1.3us/k-tile) instead of
  racing ahead and stalling.
- Last m-tile runs chunk-outer with per-head postprocess + per-head output
  DMAs so the drain tail shrinks from ~9us to ~4us.
"""
import math
import os
import sys

import numpy as np

for _p in ("/opt/trn_rl_repo", os.path.expanduser("~/.axon_site/_ro/trn_rl_repo")):
    if os.path.isdir(_p) and _p not in sys.path:
        sys.path.insert(0, _p)

import concourse.bacc as bacc
import concourse.mybir as mybir
import concourse.tile as tile
from concourse.bass_utils import run_bass_kernel_spmd

# Problem config (hardcoded from the nn.Module init)
D_MODEL = 2048
N_HEADS = 16
SEM_HD = 64
GEO_HD = 64
HEAD_DIM = 128
ROPE_DIM = 64
ROPE_HALF = ROPE_DIM // 2  # 32
ROPE_BASE = 10000.0
B, T = 4, 4096

# Sharding: 2 row groups (2 batches each) x 4 head groups (4 heads each)
N_CORES = 8
RG, HG = 2, 4
ROWS_PER_CORE = (B * T) // RG          # 8192
HEADS_PER_CORE = N_HEADS // HG         # 4
BLK = SEM_HD + GEO_HD + SEM_HD + GEO_HD + HEAD_DIM  # 384 cols per head
N_CORE = HEADS_PER_CORE * BLK          # 1536
K_TILES = D_MODEL // 128               # 16
M_TILES = ROWS_PER_CORE // 128         # 64
SLAB_MT = 2                            # m_tiles per input DMA slab
SLAB_ROWS = SLAB_MT * 128              # 256
N_SLABS = M_TILES // SLAB_MT           # 32
CHUNK = 512                            # psum bank / matmul moving size
N_CHUNKS = N_CORE // CHUNK             # 3
COS_SLOTS = T // 128                   # 32 distinct cos/sin row-tiles

_f32 = mybir.dt.float32
_bf16 = mybir.dt.bfloat16


def _build_nc():
    nc = bacc.Bacc("TRN2", target_bir_lowering=False, debug=False, num_devices=1)
    xt_d = nc.dram_tensor("xt", [D_MODEL, ROWS_PER_CORE], _bf16, kind="ExternalInput")
    w_d = nc.dram_tensor("w", [D_MODEL, N_CORE], _bf16, kind="ExternalInput")
    # host pre-permuted to the SBUF-resident layout [128, slot*half]
    cos_d = nc.dram_tensor("cos", [128, COS_SLOTS * ROPE_HALF], _f32,
                           kind="ExternalInput")
    sin_d = nc.dram_tensor("sin", [128, COS_SLOTS * ROPE_HALF], _f32,
                           kind="ExternalInput")
    qkv_d = nc.dram_tensor(
        "qkv", [3, HEADS_PER_CORE, ROWS_PER_CORE, HEAD_DIM], _f32,
        kind="ExternalOutput",
    )

    with tile.TileContext(nc) as tc:
        with (
            tc.tile_pool(name="wp", bufs=1) as wp,
            tc.tile_pool(name="xp", bufs=3) as xp,
            tc.tile_pool(name="trig", bufs=1) as trigp,
            tc.tile_pool(name="stg", bufs=3) as stgp,
            tc.tile_pool(name="tmp", bufs=2) as tmpp,
            tc.tile_pool(name="ps", bufs=2, space="PSUM") as ps,
        ):
            # DMA ring assignment: scalar carries only the slab0 pieces (plus
            # the postprocess copies later) so it never blocks on semaphore
            # recycling; sync carries W-even -> slab1 -> outputs; gpsimd
            # carries trig -> W-odd -> slabs 2+. Queue FIFO order keeps the
            # startup-critical W bytes ahead of everything non-urgent.
            xt_kd = xt_d.ap().rearrange("(k p) m -> p k m", p=128)
            slab_tiles = {}

            def load_slab(s, split=1):
                if s not in slab_tiles:
                    eng = nc.scalar if s == 0 else (nc.sync if s == 1 else nc.gpsimd)
                    t = xp.tile([128, K_TILES * SLAB_ROWS], _bf16, tag="xt")
                    view = t[:].rearrange("p (k m) -> p k m", k=K_TILES)
                    kk = K_TILES // split
                    for j in range(split):
                        eng.dma_start(
                            view[:, j * kk:(j + 1) * kk, :],
                            xt_kd[:, j * kk:(j + 1) * kk,
                                  s * SLAB_ROWS:(s + 1) * SLAB_ROWS],
                        )
                    slab_tiles[s] = t
                return slab_tiles[s]

            # First x slab split in 4 so the k=0 matmul can start after ~256KB.
            load_slab(0, split=4)

            # cos/sin tables resident (already in SBUF layout); first on the
            # gpsimd ring, ahead of the W-odd tiles.
            cos_sb = trigp.tile([128, COS_SLOTS * ROPE_HALF], _f32, tag="cos")
            nc.gpsimd.dma_start(cos_sb[:], cos_d.ap())
            sin_sb = trigp.tile([128, COS_SLOTS * ROPE_HALF], _f32, tag="sin")
            nc.gpsimd.dma_start(sin_sb[:], sin_d.ap())
            cos_v = cos_sb[:].rearrange("p (s c) -> p s c", s=COS_SLOTS)
            sin_v = sin_sb[:].rearrange("p (s c) -> p s c", s=COS_SLOTS)

            # Weights resident, one tile per k, alternating two DMA rings.
            w_kd = w_d.ap().rearrange("(k p) n -> k p n", p=128)
            w_tiles = []
            for k in range(K_TILES):
                wt = wp.tile([128, N_CORE], _bf16, tag=f"w{k}")
                eng = nc.sync if k % 2 == 0 else nc.gpsimd
                eng.dma_start(wt[:], w_kd[k])
                w_tiles.append(wt)

            # slab1 queues on sync behind the W-even tiles.
            load_slab(1)

            qkv_v = qkv_d.ap()

            def emit_mms(xt_v, i, psum, k):
                """3 chunk matmuls for (m-tile i-within-slab, k)."""
                lhs = xt_v[:, k, i * 128:(i + 1) * 128]
                for c in range(N_CHUNKS):
                    inst = nc.tensor.matmul(
                        psum[:, c * CHUNK:(c + 1) * CHUNK],
                        lhs,
                        w_tiles[k][:, c * CHUNK:(c + 1) * CHUNK],
                        start=(k == 0),
                        stop=(k == K_TILES - 1),
                    )
                    if c > 0:
                        inst.ldweights = False

            def emit_post(psum, mt, stg, hs, he):
                """RoPE + copies for heads [hs, he) of one m-tile."""
                pv = psum[:, :].rearrange(
                    "p (h t c) -> p h t c", h=HEADS_PER_CORE, t=3
                )
                sv = stg[:].rearrange(
                    "p (t h c) -> p h t c", h=HEADS_PER_CORE, t=3
                )
                nh = he - hs
                slot = mt % COS_SLOTS
                cos_bc = (
                    cos_v[:, slot, :]
                    .unsqueeze(1)
                    .unsqueeze(1)
                    .broadcast_to([128, nh, 2, ROPE_HALF])
                )
                sin_bc = (
                    sin_v[:, slot, :]
                    .unsqueeze(1)
                    .unsqueeze(1)
                    .broadcast_to([128, nh, 2, ROPE_HALF])
                )
                x1 = pv[:, hs:he, 0:2, 64:96]
                x2 = pv[:, hs:he, 0:2, 96:128]
                shp = [128, nh, 2, ROPE_HALF]
                t1 = tmpp.tile(shp, _f32, tag="t1")
                t2 = tmpp.tile(shp, _f32, tag="t2")
                t3 = tmpp.tile(shp, _f32, tag="t3")
                t4 = tmpp.tile(shp, _f32, tag="t4")
                nc.vector.tensor_mul(t1[:], x1, cos_bc)
                nc.vector.tensor_mul(t2[:], x2, sin_bc)
                nc.vector.tensor_mul(t3[:], x2, cos_bc)
                nc.vector.tensor_mul(t4[:], x1, sin_bc)
                nc.vector.tensor_sub(sv[:, hs:he, 0:2, 64:96], t1[:], t2[:])
                nc.vector.tensor_add(sv[:, hs:he, 0:2, 96:128], t3[:], t4[:])
                # sem halves of q and k
                nc.any.tensor_copy(
                    sv[:, hs:he, 0:2, 0:64], pv[:, hs:he, 0:2, 0:64]
                )
                # v
                nc.any.tensor_copy(sv[:, hs:he, 2, :], pv[:, hs:he, 2, :])

            def emit_out(stg, mt):
                m0 = mt * 128
                nc.sync.dma_start(
                    qkv_v[:, :, m0:m0 + 128, :].transpose([2, 0, 1, 3]),
                    stg[:].rearrange(
                        "p (t h c) -> p t h c", h=HEADS_PER_CORE, t=3
                    ),
                )

            def emit_out_head(stg, mt, h):
                m0 = mt * 128
                nc.sync.dma_start(
                    qkv_v[:, h, m0:m0 + 128, :].transpose([1, 0, 2]),
                    stg[:].rearrange(
                        "p (t h c) -> p t h c", h=HEADS_PER_CORE, t=3
                    )[:, :, h, :],
                )

            # ---- Startup wavefront: m-tiles 0 and 1 interleaved per k so the
            # PE consumes W k-tiles at the pace they arrive from HBM.
            xt_v0 = load_slab(0)[:].rearrange("p (k m) -> p k m", k=K_TILES)
            psum_a = ps.tile([128, N_CORE], _f32, name="psum", tag="psum")
            psum_b = ps.tile([128, N_CORE], _f32, name="psum", tag="psum")
            for k in range(K_TILES):
                emit_mms(xt_v0, 0, psum_a, k)
                emit_mms(xt_v0, 1, psum_b, k)
            for mt, psum in ((0, psum_a), (1, psum_b)):
                stg = stgp.tile([128, N_CORE], _f32, tag="stg")
                emit_post(psum, mt, stg, 0, HEADS_PER_CORE)
                emit_out(stg, mt)

            # ---- Steady state: m-tiles 2..62
            for s in range(1, N_SLABS):
                xt_sb = load_slab(s)
                xt_v = xt_sb[:].rearrange("p (k m) -> p k m", k=K_TILES)
                for i in range(SLAB_MT):
                    mt = s * SLAB_MT + i
                    if mt == M_TILES - 1:
                        break
                    psum = ps.tile([128, N_CORE], _f32, name="psum", tag="psum")
                    for k in range(K_TILES):
                        emit_mms(xt_v, i, psum, k)
                    stg = stgp.tile([128, N_CORE], _f32, tag="stg")
                    emit_post(psum, mt, stg, 0, HEADS_PER_CORE)
                    emit_out(stg, mt)

            # ---- Last m-tile: chunk-outer so heads complete progressively;
            # per-head postprocess + output DMA shrink the serial tail.
            mt = M_TILES - 1
            i = mt % SLAB_MT
            xt_v = slab_tiles[N_SLABS - 1][:].rearrange(
                "p (k m) -> p k m", k=K_TILES
            )
            psum = ps.tile([128, N_CORE], _f32, name="psum", tag="psum")
            lhs_col = lambda k: xt_v[:, k, i * 128:(i + 1) * 128]
            stg = stgp.tile([128, N_CORE], _f32, tag="stg")
            for c in range(N_CHUNKS):
                for k in range(K_TILES):
                    nc.tensor.matmul(
                        psum[:, c * CHUNK:(c + 1) * CHUNK],
                        lhs_col(k),
                        w_tiles[k][:, c * CHUNK:(c + 1) * CHUNK],
                        start=(k == 0),
                        stop=(k == K_TILES - 1),
                    )
                if c == 1:
                    # heads 0,1 live in cols 0..767 = chunks 0,1
                    emit_post(psum, mt, stg, 0, 2)
                    emit_out_head(stg, mt, 0)
                    emit_out_head(stg, mt, 1)
            emit_post(psum, mt, stg, 2, 4)
            emit_out_head(stg, mt, 2)
            emit_out_head(stg, mt, 3)

    nc.compile()
    return nc


_NC_CACHE = None
LAST_RESULTS = None


def _get_nc():
    global _NC_CACHE
    if _NC_CACHE is None:
        _NC_CACHE = _build_nc()
    return _NC_CACHE


def _host_tables(pos_offset):
    """cos/sin tables computed exactly as the reference does (f32 jax ops),
    pre-permuted to the SBUF layout [p, slot*half] with row index p holding
    positions {slot*128 + p}."""
    import jax
    import jax.numpy as jnp

    with jax.default_device(jax.devices("cpu")[0]):
        inv_freq = ROPE_BASE ** (
            -jnp.arange(0, ROPE_HALF, dtype=jnp.float32) * (2.0 / ROPE_DIM)
        )
        pos = jnp.arange(T, dtype=jnp.float32) + jnp.float32(pos_offset)
        ang = pos[:, None] * inv_freq[None, :]
        cos = np.asarray(jnp.cos(ang), dtype=np.float32)
        sin = np.asarray(jnp.sin(ang), dtype=np.float32)

    def permute(a):  # (T, half) -> (128, slots*half)
        return np.ascontiguousarray(
            a.reshape(COS_SLOTS, 128, ROPE_HALF)
            .transpose(1, 0, 2)
            .reshape(128, COS_SLOTS * ROPE_HALF)
        )

    return permute(cos), permute(sin)


def _gate(gate_logit):
    import jax
    import jax.numpy as jnp

    with jax.default_device(jax.devices("cpu")[0]):
        g = np.asarray(
            jax.nn.sigmoid(jnp.asarray(gate_logit, dtype=jnp.float32)),
            dtype=np.float32,
        )
    return g


def kernel(x, wq_sem, wk_sem, wq_geo, wk_geo, wv, gate_logit, pos_offset):
    import ml_dtypes

    bf16 = ml_dtypes.bfloat16
    x = np.asarray(x, dtype=np.float32)
    wq_sem = np.asarray(wq_sem, dtype=np.float32)
    wk_sem = np.asarray(wk_sem, dtype=np.float32)
    wq_geo = np.asarray(wq_geo, dtype=np.float32)
    wk_geo = np.asarray(wk_geo, dtype=np.float32)
    wv = np.asarray(wv, dtype=np.float32)
    pos_off = int(np.asarray(pos_offset))

    g = _gate(gate_logit)  # (16,)
    sem_scale = np.float32(1.0 / math.sqrt(float(SEM_HD)))
    geo_scale = np.float32(1.0 / math.sqrt(float(GEO_HD)))
    q_sem_col = (np.float32(2.0) * g * sem_scale).astype(np.float32)   # per head
    q_geo_col = ((np.float32(2.0) - np.float32(2.0) * g) * geo_scale).astype(
        np.float32
    )

    # Per-core weight slabs, cols per head: [qsem|qgeo|ksem|kgeo|v]
    w_cores = []
    for hg in range(HG):
        cols = []
        for hl in range(HEADS_PER_CORE):
            h = hg * HEADS_PER_CORE + hl
            cols.append(wq_sem[:, h * 64:(h + 1) * 64] * q_sem_col[h])
            cols.append(wq_geo[:, h * 64:(h + 1) * 64] * q_geo_col[h])
            cols.append(wk_sem[:, h * 64:(h + 1) * 64])
            cols.append(wk_geo[:, h * 64:(h + 1) * 64])
            cols.append(wv[:, h * 128:(h + 1) * 128])
        w_cores.append(np.concatenate(cols, axis=1).astype(bf16))

    # x^T in bf16, split into the two row groups
    xt = x.reshape(B * T, D_MODEL).T.astype(bf16)  # (2048, 16384) C-contig copy
    xt_rg = [
        np.ascontiguousarray(xt[:, rg * ROWS_PER_CORE:(rg + 1) * ROWS_PER_CORE])
        for rg in range(RG)
    ]

    cos, sin = _host_tables(pos_off)

    in_maps = []
    for core in range(N_CORES):
        rg, hg = core // HG, core % HG
        in_maps.append(
            {"xt": xt_rg[rg], "w": w_cores[hg], "cos": cos, "sin": sin}
        )

    nc = _get_nc()
    res = run_bass_kernel_spmd(nc, in_maps, list(range(N_CORES)))
    global LAST_RESULTS
    LAST_RESULTS = res

    q_cat = np.empty((B, N_HEADS, T, HEAD_DIM), np.float32)
    k_cat = np.empty((B, N_HEADS, T, HEAD_DIM), np.float32)
    vh = np.empty((B, N_HEADS, T, HEAD_DIM), np.float32)
    for core in range(N_CORES):
        rg, hg = core // HG, core % HG
        a = res.results[core]["qkv"]  # (3, 4, 8192, 128)
        for t3_idx, dst in ((0, q_cat), (1, k_cat), (2, vh)):
            # (4, 8192, 128) -> (heads, b_local, T, 128)
            b = a[t3_idx].reshape(HEADS_PER_CORE, 2, T, HEAD_DIM)
            dst[
                rg * 2:(rg + 1) * 2,
                hg * HEADS_PER_CORE:(hg + 1) * HEADS_PER_CORE,
            ] = b.transpose(1, 0, 2, 3)
    return q_cat, k_cat, vh


# revision 9
# speedup vs baseline: 1.2136x; 1.0110x over previous
"""Trainium2 Bass kernel for nn_DecoupledAttentionWeight.

Computes the five projections q_sem/k_sem/q_geo/k_geo/v of x, applies RoPE to
the geo paths, the per-head sigmoid gate + per-path scaling to q (folded into
the projection weights host-side), and returns (q_cat, k_cat, vh) shaped
(B, H, T, 128) each.

Sharding over 8 NeuronCores: 2-way data-parallel over batch (batches {0,1} /
{2,3}) x 4-way tensor-parallel over heads (4 heads per core). Each core runs
one big [8192 x 2048] @ [2048 x 1536] matmul in bf16 (full PE speed, ~2.3e-3
rel err, well within tolerance) with the per-head output columns packed as
[q_sem|q_geo|k_sem|k_geo|v] so the sem||geo concat is free, then RoPE on the
geo strips via DVE with broadcast access patterns.

v5: v2's plain m-tile loop (the v3/v4 startup wavefront regressed: cross-ring
k-interleaving made W arrival pacing fragile and the stalls re-throttled HAM),
plus: host-pre-permuted cos/sin (cheap descriptors), slab0 split in 8,
copy-psum-to-staging-first postprocess (psum freed ~1.2us after the last MM
of an m-tile, decoupling the PE stream from the RoPE chain), per-head
pipelined postprocess + output DMA on the final m-tile.
"""
import math
import os
import sys

import numpy as np

for _p in ("/opt/trn_rl_repo", os.path.expanduser("~/.axon_site/_ro/trn_rl_repo")):
    if os.path.isdir(_p) and _p not in sys.path:
        sys.path.insert(0, _p)

import concourse.bacc as bacc
import concourse.mybir as mybir
import concourse.tile as tile
from concourse.bass_utils import run_bass_kernel_spmd

# Problem config (hardcoded from the nn.Module init)
D_MODEL = 2048
N_HEADS = 16
SEM_HD = 64
GEO_HD = 64
HEAD_DIM = 128
ROPE_DIM = 64
ROPE_HALF = ROPE_DIM // 2  # 32
ROPE_BASE = 10000.0
B, T = 4, 4096

# Sharding: 2 row groups (2 batches each) x 4 head groups (4 heads each)
N_CORES = 8
RG, HG = 2, 4
ROWS_PER_CORE = (B * T) // RG          # 8192
HEADS_PER_CORE = N_HEADS // HG         # 4
BLK = SEM_HD + GEO_HD + SEM_HD + GEO_HD + HEAD_DIM  # 384 cols per head
N_CORE = HEADS_PER_CORE * BLK          # 1536
K_TILES = D_MODEL // 128               # 16
M_TILES = ROWS_PER_CORE // 128         # 64
SLAB_MT = 2                            # m_tiles per input DMA slab
SLAB_ROWS = SLAB_MT * 128              # 256
N_SLABS = M_TILES // SLAB_MT           # 32
CHUNK = 512                            # psum bank / matmul moving size
N_CHUNKS = N_CORE // CHUNK             # 3
COS_SLOTS = T // 128                   # 32 distinct cos/sin row-tiles

_f32 = mybir.dt.float32
_bf16 = mybir.dt.bfloat16


def _build_nc():
    nc = bacc.Bacc("TRN2", target_bir_lowering=False, debug=False, num_devices=1)
    xt_d = nc.dram_tensor("xt", [D_MODEL, ROWS_PER_CORE], _bf16, kind="ExternalInput")
    w_d = nc.dram_tensor("w", [D_MODEL, N_CORE], _bf16, kind="ExternalInput")
    # host pre-permuted to the SBUF-resident layout [128, slot*half]
    cos_d = nc.dram_tensor("cos", [128, COS_SLOTS * ROPE_HALF], _f32,
                           kind="ExternalInput")
    sin_d = nc.dram_tensor("sin", [128, COS_SLOTS * ROPE_HALF], _f32,
                           kind="ExternalInput")
    qkv_d = nc.dram_tensor(
        "qkv", [3, HEADS_PER_CORE, ROWS_PER_CORE, HEAD_DIM], _f32,
        kind="ExternalOutput",
    )

    with tile.TileContext(nc) as tc:
        with (
            tc.tile_pool(name="wp", bufs=1) as wp,
            tc.tile_pool(name="xp", bufs=3) as xp,
            tc.tile_pool(name="trig", bufs=1) as trigp,
            tc.tile_pool(name="stg", bufs=3) as stgp,
            tc.tile_pool(name="tmp", bufs=2) as tmpp,
            tc.tile_pool(name="ps", bufs=2, space="PSUM") as ps,
        ):
            xt_kd = xt_d.ap().rearrange("(k p) m -> p k m", p=128)
            slab_tiles = {}

            def load_slab(s, split=1):
                if s not in slab_tiles:
                    t = xp.tile([128, K_TILES * SLAB_ROWS], _bf16, tag="xt")
                    view = t[:].rearrange("p (k m) -> p k m", k=K_TILES)
                    kk = K_TILES // split
                    for j in range(split):
                        # scalar HWDGE ring: parallel to W/output DMAs
                        nc.scalar.dma_start(
                            view[:, j * kk:(j + 1) * kk, :],
                            xt_kd[:, j * kk:(j + 1) * kk,
                                  s * SLAB_ROWS:(s + 1) * SLAB_ROWS],
                        )
                    slab_tiles[s] = t
                return slab_tiles[s]

            # First x slab split in 8 so the k=0 matmul can start after ~128KB.
            load_slab(0, split=8)

            # Weights resident, one tile per k, alternating two DMA rings.
            w_kd = w_d.ap().rearrange("(k p) n -> k p n", p=128)
            w_tiles = []
            for k in range(K_TILES):
                wt = wp.tile([128, N_CORE], _bf16, tag=f"w{k}")
                eng = nc.sync if k % 2 == 0 else nc.gpsimd
                eng.dma_start(wt[:], w_kd[k])
                w_tiles.append(wt)

            # cos/sin tables resident (already in SBUF layout); cheap
            # large-descriptor DMAs on the scalar ring behind the slab pieces.
            cos_sb = trigp.tile([128, COS_SLOTS * ROPE_HALF], _f32, tag="cos")
            nc.scalar.dma_start(cos_sb[:], cos_d.ap())
            sin_sb = trigp.tile([128, COS_SLOTS * ROPE_HALF], _f32, tag="sin")
            nc.scalar.dma_start(sin_sb[:], sin_d.ap())
            cos_v = cos_sb[:].rearrange("p (s c) -> p s c", s=COS_SLOTS)
            sin_v = sin_sb[:].rearrange("p (s c) -> p s c", s=COS_SLOTS)

            qkv_v = qkv_d.ap()

            def emit_post(psum, mt, stg, hs, he):
                """Copy psum->staging (freeing psum fast), then RoPE the geo
                strips in place in staging, for heads [hs, he)."""
                pv = psum[:, :].rearrange(
                    "p (h t c) -> p h t c", h=HEADS_PER_CORE, t=3
                )
                sv = stg[:].rearrange(
                    "p (t h c) -> p h t c", h=HEADS_PER_CORE, t=3
                )
                # q and k blocks (contain the RoPE strips) on DVE, v on ACT;
                # after these three copies the psum tile is free.
                nc.vector.tensor_copy(sv[:, hs:he, 0, :], pv[:, hs:he, 0, :])
                nc.vector.tensor_copy(sv[:, hs:he, 1, :], pv[:, hs:he, 1, :])
                nc.scalar.copy(sv[:, hs:he, 2, :], pv[:, hs:he, 2, :])

                nh = he - hs
                slot = mt % COS_SLOTS
                cos_bc = (
                    cos_v[:, slot, :]
                    .unsqueeze(1)
                    .unsqueeze(1)
                    .broadcast_to([128, nh, 2, ROPE_HALF])
                )
                sin_bc = (
                    sin_v[:, slot, :]
                    .unsqueeze(1)
                    .unsqueeze(1)
                    .broadcast_to([128, nh, 2, ROPE_HALF])
                )
                x1 = sv[:, hs:he, 0:2, 64:96]
                x2 = sv[:, hs:he, 0:2, 96:128]
                shp = [128, nh, 2, ROPE_HALF]
                t1 = tmpp.tile(shp, _f32, tag="t1")
                t2 = tmpp.tile(shp, _f32, tag="t2")
                t3 = tmpp.tile(shp, _f32, tag="t3")
                t4 = tmpp.tile(shp, _f32, tag="t4")
                nc.vector.tensor_mul(t1[:], x1, cos_bc)
                nc.vector.tensor_mul(t2[:], x2, sin_bc)
                nc.vector.tensor_mul(t3[:], x2, cos_bc)
                nc.vector.tensor_mul(t4[:], x1, sin_bc)
                nc.vector.tensor_sub(sv[:, hs:he, 0:2, 64:96], t1[:], t2[:])
                nc.vector.tensor_add(sv[:, hs:he, 0:2, 96:128], t3[:], t4[:])

            def emit_out(stg, mt):
                m0 = mt * 128
                nc.sync.dma_start(
                    qkv_v[:, :, m0:m0 + 128, :].transpose([2, 0, 1, 3]),
                    stg[:].rearrange(
                        "p (t h c) -> p t h c", h=HEADS_PER_CORE, t=3
                    ),
                )

            def emit_out_head(stg, mt, h):
                m0 = mt * 128
                nc.sync.dma_start(
                    qkv_v[:, h, m0:m0 + 128, :].transpose([1, 0, 2]),
                    stg[:].rearrange(
                        "p (t h c) -> p t h c", h=HEADS_PER_CORE, t=3
                    )[:, :, h, :],
                )

            # m-tiles 0..62: k-outer / chunk-inner with stationary reuse
            for s in range(N_SLABS):
                xt_sb = load_slab(s)
                xt_v = xt_sb[:].rearrange("p (k m) -> p k m", k=K_TILES)
                for i in range(SLAB_MT):
                    mt = s * SLAB_MT + i
                    if mt == M_TILES - 1:
                        break
                    psum = ps.tile([128, N_CORE], _f32, name="psum", tag="psum")
                    for k in range(K_TILES):
                        lhs = xt_v[:, k, i * 128:(i + 1) * 128]
                        for c in range(N_CHUNKS):
                            inst = nc.tensor.matmul(
                                psum[:, c * CHUNK:(c + 1) * CHUNK],
                                lhs,
                                w_tiles[k][:, c * CHUNK:(c + 1) * CHUNK],
                                start=(k == 0),
                                stop=(k == K_TILES - 1),
                            )
                            if c > 0:
                                inst.ldweights = False
                    stg = stgp.tile([128, N_CORE], _f32, tag="stg")
                    emit_post(psum, mt, stg, 0, HEADS_PER_CORE)
                    emit_out(stg, mt)

            # Last m-tile: chunk-outer so heads complete progressively;
            # per-head postprocess + output DMA shrink the serial tail.
            mt = M_TILES - 1
            i = mt % SLAB_MT
            xt_v = slab_tiles[N_SLABS - 1][:].rearrange(
                "p (k m) -> p k m", k=K_TILES
            )
            psum = ps.tile([128, N_CORE], _f32, name="psum", tag="psum")
            stg = stgp.tile([128, N_CORE], _f32, tag="stg")
            for c in range(N_CHUNKS):
                for k in range(K_TILES):
                    nc.tensor.matmul(
                        psum[:, c * CHUNK:(c + 1) * CHUNK],
                        xt_v[:, k, i * 128:(i + 1) * 128],
                        w_tiles[k][:, c * CHUNK:(c + 1) * CHUNK],
                        start=(k == 0),
                        stop=(k == K_TILES - 1),
                    )
                if c == 1:
                    # heads 0,1 live in cols 0..767 = chunks 0,1
                    emit_post(psum, mt, stg, 0, 2)
                    emit_out_head(stg, mt, 0)
                    emit_out_head(stg, mt, 1)
            emit_post(psum, mt, stg, 2, 4)
            emit_out_head(stg, mt, 2)
            emit_out_head(stg, mt, 3)

    nc.compile()
    return nc


_NC_CACHE = None
LAST_RESULTS = None


def _get_nc():
    global _NC_CACHE
    if _NC_CACHE is None:
        _NC_CACHE = _build_nc()
    return _NC_CACHE


def _host_tables(pos_offset):
    """cos/sin tables computed exactly as the reference does (f32 jax ops),
    pre-permuted to the SBUF layout [p, slot*half] with row index p holding
    positions {slot*128 + p}."""
    import jax
    import jax.numpy as jnp

    with jax.default_device(jax.devices("cpu")[0]):
        inv_freq = ROPE_BASE ** (
            -jnp.arange(0, ROPE_HALF, dtype=jnp.float32) * (2.0 / ROPE_DIM)
        )
        pos = jnp.arange(T, dtype=jnp.float32) + jnp.float32(pos_offset)
        ang = pos[:, None] * inv_freq[None, :]
        cos = np.asarray(jnp.cos(ang), dtype=np.float32)
        sin = np.asarray(jnp.sin(ang), dtype=np.float32)

    def permute(a):  # (T, half) -> (128, slots*half)
        return np.ascontiguousarray(
            a.reshape(COS_SLOTS, 128, ROPE_HALF)
            .transpose(1, 0, 2)
            .reshape(128, COS_SLOTS * ROPE_HALF)
        )

    return permute(cos), permute(sin)


def _gate(gate_logit):
    import jax
    import jax.numpy as jnp

    with jax.default_device(jax.devices("cpu")[0]):
        g = np.asarray(
            jax.nn.sigmoid(jnp.asarray(gate_logit, dtype=jnp.float32)),
            dtype=np.float32,
        )
    return g


def kernel(x, wq_sem, wk_sem, wq_geo, wk_geo, wv, gate_logit, pos_offset):
    import ml_dtypes

    bf16 = ml_dtypes.bfloat16
    x = np.asarray(x, dtype=np.float32)
    wq_sem = np.asarray(wq_sem, dtype=np.float32)
    wk_sem = np.asarray(wk_sem, dtype=np.float32)
    wq_geo = np.asarray(wq_geo, dtype=np.float32)
    wk_geo = np.asarray(wk_geo, dtype=np.float32)
    wv = np.asarray(wv, dtype=np.float32)
    pos_off = int(np.asarray(pos_offset))

    g = _gate(gate_logit)  # (16,)
    sem_scale = np.float32(1.0 / math.sqrt(float(SEM_HD)))
    geo_scale = np.float32(1.0 / math.sqrt(float(GEO_HD)))
    q_sem_col = (np.float32(2.0) * g * sem_scale).astype(np.float32)   # per head
    q_geo_col = ((np.float32(2.0) - np.float32(2.0) * g) * geo_scale).astype(
        np.float32
    )

    # Per-core weight slabs, cols per head: [qsem|qgeo|ksem|kgeo|v]
    w_cores = []
    for hg in range(HG):
        cols = []
        for hl in range(HEADS_PER_CORE):
            h = hg * HEADS_PER_CORE + hl
            cols.append(wq_sem[:, h * 64:(h + 1) * 64] * q_sem_col[h])
            cols.append(wq_geo[:, h * 64:(h + 1) * 64] * q_geo_col[h])
            cols.append(wk_sem[:, h * 64:(h + 1) * 64])
            cols.append(wk_geo[:, h * 64:(h + 1) * 64])
            cols.append(wv[:, h * 128:(h + 1) * 128])
        w_cores.append(np.concatenate(cols, axis=1).astype(bf16))

    # x^T in bf16, split into the two row groups
    xt = x.reshape(B * T, D_MODEL).T.astype(bf16)  # (2048, 16384) C-contig copy
    xt_rg = [
        np.ascontiguousarray(xt[:, rg * ROWS_PER_CORE:(rg + 1) * ROWS_PER_CORE])
        for rg in range(RG)
    ]

    cos, sin = _host_tables(pos_off)

    in_maps = []
    for core in range(N_CORES):
        rg, hg = core // HG, core % HG
        in_maps.append(
            {"xt": xt_rg[rg], "w": w_cores[hg], "cos": cos, "sin": sin}
        )

    nc = _get_nc()
    res = run_bass_kernel_spmd(nc, in_maps, list(range(N_CORES)))
    global LAST_RESULTS
    LAST_RESULTS = res

    q_cat = np.empty((B, N_HEADS, T, HEAD_DIM), np.float32)
    k_cat = np.empty((B, N_HEADS, T, HEAD_DIM), np.float32)
    vh = np.empty((B, N_HEADS, T, HEAD_DIM), np.float32)
    for core in range(N_CORES):
        rg, hg = core // HG, core % HG
        a = res.results[core]["qkv"]  # (3, 4, 8192, 128)
        for t3_idx, dst in ((0, q_cat), (1, k_cat), (2, vh)):
            # (4, 8192, 128) -> (heads, b_local, T, 128)
            b = a[t3_idx].reshape(HEADS_PER_CORE, 2, T, HEAD_DIM)
            dst[
                rg * 2:(rg + 1) * 2,
                hg * HEADS_PER_CORE:(hg + 1) * HEADS_PER_CORE,
            ] = b.transpose(1, 0, 2, 3)
    return q_cat, k_cat, vh
